# revision 1
# baseline (speedup 1.0000x reference)
"""Multi-head self-attention (B=2, T=2048, C=1024, H=16) on 8 NeuronCores.

Sharding: core c -> (batch b = c//4, head-group g = c%4); each core computes
4 heads' attention for one batch plus its slice of the QKV/out projections.
Per-core partial outputs (over head groups) are summed on the host.

Device-side layout is fully transposed (feature dim on partitions):
  xt [C, T] -> QT/KT [256, T] (j on partitions), V natural [T, 256],
  ST = K Qt (scores transposed, tk on partitions).
The stationary PV operand is V extended with 64 columns of ones, so the
yext accumulator's rows 64..127 all hold the softmax denominator — a free
hardware broadcast that lets normalization run entirely on VectorE
(reciprocal + multiply) without touching the PE or ScalarE.
Exp runs on ScalarE over 2-PSUM-bank tiles to amortize the ~352-cycle
per-instruction overhead. Matmul operands are fp16 (10-bit mantissa,
~6e-4 end-to-end rel err) with fp32 PSUM accumulation. bq/bk (and the
1/sqrt(hd) scale) are folded in on the host; bv/bo commute through softmax
(rows sum to 1) and are added on the host at the end.
"""

import numpy as np

import concourse.bacc as bacc
import concourse.mybir as mybir
import concourse.tile as tile
from concourse.bass_utils import run_bass_kernel_spmd

B, T, C, H = 2, 2048, 1024, 16
HD = C // H  # 64
NCORES = 8
GROUPS = 4  # head groups (one per core within a batch)
HPG = H // GROUPS  # heads per group = 4
JW = HPG * HD  # per-core projection slice width = 256

F32 = mybir.dt.float32
MMDT = mybir.dt.float16
NPDT = np.float16

_CACHED_NC = None


def _outproj(nc, psA, outp, yt_sb, wo_sb, out, qb):
    for tt in range(qb * 4, (qb + 1) * 4):
        po = psA.tile([128, 1024], F32, tag="mm", name="po")
        for mb in range(2):
            for jt in range(2):
                nc.tensor.matmul(
                    po[:, mb * 512 : (mb + 1) * 512],
                    yt_sb[:, jt, tt * 128 : (tt + 1) * 128],
                    wo_sb[:, jt, mb * 512 : (mb + 1) * 512],
                    start=(jt == 0),
                    stop=(jt == 1),
                )
        ob = outp.tile([128, 1024], F32, tag="ob", name="ob")
        nc.vector.tensor_copy(out=ob[:], in_=po[:])
        nc.sync.dma_start(out=out[tt * 128 : (tt + 1) * 128, :], in_=ob[:])


def _build():
    nc = bacc.Bacc("TRN2", target_bir_lowering=False, num_devices=NCORES)

    xt = nc.dram_tensor("xt", [C, T], MMDT, kind="ExternalInput")
    wq = nc.dram_tensor("wq", [C, JW], MMDT, kind="ExternalInput")
    wk = nc.dram_tensor("wk", [C, JW], MMDT, kind="ExternalInput")
    wv = nc.dram_tensor("wv", [C, JW], MMDT, kind="ExternalInput")
    wo = nc.dram_tensor("wo", [JW, C], MMDT, kind="ExternalInput")
    bq = nc.dram_tensor("bq", [JW], F32, kind="ExternalInput")
    bk = nc.dram_tensor("bk", [JW], F32, kind="ExternalInput")
    out = nc.dram_tensor("out", [T, C], F32, kind="ExternalOutput")

    xt_ap = xt[:, :].rearrange("(cc p) t -> p cc t", p=128)  # [128, 8, T]
    wq_ap = wq[:, :].rearrange("(cc p) j -> p cc j", p=128)  # [128, 8, 256]
    wk_ap = wk[:, :].rearrange("(cc p) j -> p cc j", p=128)
    wv_ap = wv[:, :].rearrange("(cc p) j -> p cc j", p=128)
    wo_ap = wo[:, :].rearrange("(jt p) m -> p jt m", p=128)  # [128, 2, C]
    bq_ap = bq[:].rearrange("(jt p) -> p jt", p=128)  # [128, 2]
    bk_ap = bk[:].rearrange("(jt p) -> p jt", p=128)

    with tile.TileContext(nc) as tc:
        with (
            tc.tile_pool(name="big", bufs=1) as big,
            tc.tile_pool(name="work", bufs=8) as work,
            tc.tile_pool(name="nrm", bufs=4) as nrm,
            tc.tile_pool(name="outp", bufs=3) as outp,
            tc.tile_pool(name="psA", bufs=3, space="PSUM") as psA,
            tc.tile_pool(name="psY", bufs=2, space="PSUM") as psY,
        ):
            # ---- persistent SBUF tensors ----
            xt_sb = big.tile([128, 8, T], MMDT)
            wq_sb = big.tile([128, 8, JW], MMDT)
            wk_sb = big.tile([128, 8, JW], MMDT)
            wv_sb = big.tile([128, 8, JW], MMDT)
            wo_sb = big.tile([128, 2, C], MMDT)
            qt_sb = big.tile([128, 2, T], MMDT)
            kt_sb = big.tile([128, 2, T], MMDT)
            yt_sb = big.tile([128, 2, T], MMDT)
            # V natural + 64 ones columns per head (denominator broadcast rows)
            v_sb = big.tile([128, 16, HPG, 128], MMDT)
            bq_sb = big.tile([128, 2], F32)
            bk_sb = big.tile([128, 2], F32)

            # FIFO per HWDGE ring: order loads by first use
            nc.sync.dma_start(out=wq_sb[:], in_=wq_ap)
            nc.sync.dma_start(out=xt_sb[:, 0, :], in_=xt_ap[:, 0, :])
            nc.sync.dma_start(out=wk_sb[:], in_=wk_ap)
            nc.sync.dma_start(out=bq_sb[:], in_=bq_ap)
            nc.sync.dma_start(out=bk_sb[:], in_=bk_ap)
            nc.vector.memset(v_sb[:, :, :, HD:128], 1.0)
            for cc in range(1, 8):
                nc.sync.dma_start(out=xt_sb[:, cc, :], in_=xt_ap[:, cc, :])
            nc.sync.dma_start(out=wv_sb[:], in_=wv_ap)
            nc.sync.dma_start(out=wo_sb[:], in_=wo_ap)

            # ---- phase 1a: QT, KT for jt=0 ----
            def _qk(jt):
                for tb in range(2):
                    ts = slice(tb * 1024, (tb + 1) * 1024)
                    pq = psA.tile([128, 1024], F32, tag="mm", name="pq")
                    pk = psA.tile([128, 1024], F32, tag="mm", name="pk")
                    for half in range(2):
                        hs = slice(half * 512, (half + 1) * 512)
                        xs = slice(tb * 1024 + half * 512, tb * 1024 + half * 512 + 512)
                        for cc in range(8):
                            nc.tensor.matmul(
                                pq[:, hs],
                                wq_sb[:, cc, jt * 128 : (jt + 1) * 128],
                                xt_sb[:, cc, xs],
                                start=(cc == 0),
                                stop=(cc == 7),
                            )
                        for cc in range(8):
                            nc.tensor.matmul(
                                pk[:, hs],
                                wk_sb[:, cc, jt * 128 : (jt + 1) * 128],
                                xt_sb[:, cc, xs],
                                start=(cc == 0),
                                stop=(cc == 7),
                            )
                    nc.vector.tensor_scalar_add(
                        out=qt_sb[:, jt, ts], in0=pq[:], scalar1=bq_sb[:, jt : jt + 1]
                    )
                    nc.vector.tensor_scalar_add(
                        out=kt_sb[:, jt, ts], in0=pk[:], scalar1=bk_sb[:, jt : jt + 1]
                    )

            def _qk_halfchain(jt, tb, half, which):
                w_sb, b_sb, o_sb = (
                    (wq_sb, bq_sb, qt_sb) if which == "q" else (wk_sb, bk_sb, kt_sb)
                )
                hs = slice(tb * 1024 + half * 512, tb * 1024 + half * 512 + 512)
                p1 = psA.tile([128, 512], F32, tag="mm", name="p1")
                for cc in range(8):
                    nc.tensor.matmul(
                        p1[:],
                        w_sb[:, cc, jt * 128 : (jt + 1) * 128],
                        xt_sb[:, cc, hs],
                        start=(cc == 0),
                        stop=(cc == 7),
                    )
                nc.vector.tensor_scalar_add(
                    out=o_sb[:, jt, hs], in0=p1[:], scalar1=b_sb[:, jt : jt + 1]
                )

            def _vproj():
                for tg in range(8):  # V: two t-chunks of 128 per psum tile
                    pv = psA.tile([128, 1024], F32, tag="mm", name="pv")
                    for half in range(2):
                        tt = tg * 2 + half
                        for cc in range(8):
                            nc.tensor.matmul(
                                pv[:, half * 512 : half * 512 + JW],
                                xt_sb[:, cc, tt * 128 : (tt + 1) * 128],
                                wv_sb[:, cc, :],
                                start=(cc == 0),
                                stop=(cc == 7),
                            )
                    pv3 = pv[:].rearrange("p (half j) -> p half j", half=2)
                    nc.vector.tensor_copy(
                        out=v_sb[:, tg * 2 : tg * 2 + 2, :, 0:HD],
                        in_=pv3[:, :, 0:JW].rearrange(
                            "p half (h d) -> p half h d", h=HPG
                        ),
                    )

            def _attn(h, qb):
                jt, pb = h // 2, 64 * (h % 2)
                qs = slice(qb * 512, (qb + 1) * 512)
                yext = psY.tile([128, 512], F32, tag="yext", name="yext")
                for kg in range(8):  # pairs of tk chunks
                    st = psA.tile([128, 1024], F32, tag="mm", name="st")
                    for half in range(2):
                        kc = kg * 2 + half
                        nc.tensor.matmul(
                            st[:, half * 512 : (half + 1) * 512],
                            kt_sb[pb : pb + HD, jt, kc * 128 : (kc + 1) * 128],
                            qt_sb[pb : pb + HD, jt, qs],
                            start=True,
                            stop=True,
                        )
                    es = work.tile([128, 1024], MMDT, tag="es", name="es")
                    nc.scalar.activation(
                        out=es[:], in_=st[:], func=mybir.ActivationFunctionType.Exp
                    )
                    for half in range(2):
                        kc = kg * 2 + half
                        nc.tensor.matmul(
                            yext[:],
                            v_sb[:, kc, h, :],
                            es[:, half * 512 : (half + 1) * 512],
                            start=(kc == 0),
                            stop=(kc == 15),
                        )
                r32 = nrm.tile([HD, 512], F32, tag="r32", name="r32")
                nc.vector.reciprocal(out=r32[:], in_=yext[HD:128, :])
                nc.vector.tensor_mul(
                    out=yt_sb[pb : pb + HD, jt, qs], in0=r32[:], in1=yext[0:HD, :]
                )

            _qk(0)
            _qk(1)
            _vproj()
            for h in range(HPG):
                for qb in range(4):
                    _attn(h, qb)

            # ---- phase 3: out projection ----
            for qb in range(4):
                _outproj(nc, psA, outp, yt_sb, wo_sb, out, qb)

    nc.finalize()
    return nc


def _get_nc():
    global _CACHED_NC
    if _CACHED_NC is None:
        _CACHED_NC = _build()
    return _CACHED_NC


def make_in_maps(x, Wq, bq, Wk, bk, Wv, Wo):
    """Per-core input dicts (host-side sharding + layout + fp16 cast)."""
    xts = [
        np.ascontiguousarray(np.asarray(x[b], np.float32).T).astype(NPDT)
        for b in range(B)
    ]
    wq_f = np.asarray(Wq, np.float32) / 8.0
    wk_f = np.asarray(Wk, np.float32)
    wv_f = np.asarray(Wv, np.float32)
    wo_f = np.asarray(Wo, np.float32)
    bq_f = np.asarray(bq, np.float32) / 8.0
    bk_f = np.asarray(bk, np.float32)
    in_maps = []
    for c in range(NCORES):
        b, g = c // GROUPS, c % GROUPS
        js = slice(g * JW, (g + 1) * JW)
        in_maps.append(
            {
                "xt": xts[b],
                "wq": np.ascontiguousarray(wq_f[:, js]).astype(NPDT),
                "wk": np.ascontiguousarray(wk_f[:, js]).astype(NPDT),
                "wv": np.ascontiguousarray(wv_f[:, js]).astype(NPDT),
                "wo": np.ascontiguousarray(wo_f[js, :]).astype(NPDT),
                "bq": np.ascontiguousarray(bq_f[js]),
                "bk": np.ascontiguousarray(bk_f[js]),
            }
        )
    return in_maps


def combine(results, bias_row):
    """Sum per-core head-group partials and add the host-side bias row."""
    out = np.zeros((B, T, C), np.float32)
    for c in range(NCORES):
        out[c // GROUPS] += results[c]["out"]
    out += bias_row
    return out


def kernel(x, Wq, bq, Wk, bk, Wv, bv, Wo, bo):
    nc = _get_nc()
    in_maps = make_in_maps(x, Wq, bq, Wk, bk, Wv, Wo)
    res = run_bass_kernel_spmd(nc, in_maps, core_ids=list(range(NCORES)))
    bias_row = (
        np.asarray(bv, np.float32) @ np.asarray(Wo, np.float32)
        + np.asarray(bo, np.float32)
    ).astype(np.float32)
    return combine(res.results, bias_row)



# revision 6
# speedup vs baseline: 1.0400x; 1.0400x over previous
"""Multi-head self-attention (B=2, T=2048, C=1024, H=16) on 8 NeuronCores.

Sharding: core c -> (batch b = c//4, head-group g = c%4); each core computes
4 heads' attention for one batch plus its slice of the QKV/out projections.
Per-core partial outputs (over head groups) are summed on the host.

Device-side layout is fully transposed (feature dim on partitions):
  xt [C, T] -> QT/KT [256, T] (j on partitions), V natural [T, 256],
  ST = K Qt (scores transposed, tk on partitions).
The stationary PV operand is V extended with 64 columns of ones, so the
yext accumulator's rows 64..127 all hold the softmax denominator — a free
hardware broadcast; normalization = reciprocal_approx_fast + multiply on DVE.

Schedule: the ScalarE exp stream (128 x ~1.07us, unsplittable to any other
engine) is the kernel's critical path. Emission order starts the first
score tile ~12us in (x DMA'd in 4 T-chunks, K proj per chunk) and threads
all remaining PE work (V proj, K jt1, Q blocks, out-projection) into the
exp-paced attention stream as fillers, so PE slack hides under exp.
Iteration order is qb-major; out-projection for q-block qb is emitted
during qb+1, output stored fp16 to halve the store traffic.
"""

import numpy as np

import concourse.bacc as bacc
import concourse.mybir as mybir
import concourse.tile as tile
from concourse.bass_utils import run_bass_kernel_spmd

B, T, C, H = 2, 2048, 1024, 16
HD = C // H  # 64
NCORES = 8
GROUPS = 4  # head groups (one per core within a batch)
HPG = H // GROUPS  # heads per group = 4
JW = HPG * HD  # per-core projection slice width = 256

F32 = mybir.dt.float32
MMDT = mybir.dt.float16
NPDT = np.float16

_CACHED_NC = None


def _build(debug=False):
    nc = bacc.Bacc("TRN2", target_bir_lowering=False, num_devices=NCORES)

    xt = nc.dram_tensor("xt", [C, T], MMDT, kind="ExternalInput")
    wq = nc.dram_tensor("wq", [C, JW], MMDT, kind="ExternalInput")
    wk = nc.dram_tensor("wk", [C, JW], MMDT, kind="ExternalInput")
    wv = nc.dram_tensor("wv", [C, JW], MMDT, kind="ExternalInput")
    wo = nc.dram_tensor("wo", [JW, C], MMDT, kind="ExternalInput")
    bq = nc.dram_tensor("bq", [JW], F32, kind="ExternalInput")
    bk = nc.dram_tensor("bk", [JW], F32, kind="ExternalInput")
    out = nc.dram_tensor("out", [T, C], MMDT, kind="ExternalOutput")

    xt_ap = xt[:, :].rearrange("(cc p) t -> p cc t", p=128)  # [128, 8, T]
    wq_ap = wq[:, :].rearrange("(cc p) j -> p cc j", p=128)  # [128, 8, 256]
    wk_ap = wk[:, :].rearrange("(cc p) j -> p cc j", p=128)
    wv_ap = wv[:, :].rearrange("(cc p) j -> p cc j", p=128)
    wo_ap = wo[:, :].rearrange("(jt p) m -> p jt m", p=128)  # [128, 2, C]
    bq_ap = bq[:].rearrange("(jt p) -> p jt", p=128)  # [128, 2]
    bk_ap = bk[:].rearrange("(jt p) -> p jt", p=128)

    with tile.TileContext(nc) as tc:
        with (
            tc.tile_pool(name="big", bufs=1) as big,
            tc.tile_pool(name="work", bufs=18) as work,
            tc.tile_pool(name="nrm", bufs=4) as nrm,
            tc.tile_pool(name="outp", bufs=4) as outp,
            tc.tile_pool(name="psA", bufs=3, space="PSUM") as psA,
            tc.tile_pool(name="psY", bufs=2, space="PSUM") as psY,
        ):
            # ---- persistent SBUF tensors ----
            xt_sb = big.tile([128, 8, T], MMDT)
            wq_sb = big.tile([128, 8, JW], MMDT)
            wk_sb = big.tile([128, 8, JW], MMDT)
            wv_sb = big.tile([128, 8, JW], MMDT)
            wo_sb = big.tile([128, 2, C], MMDT)
            qt_sb = big.tile([128, 2, T], MMDT)
            kt_sb = big.tile([128, 2, T], MMDT)
            yt_sb = big.tile([128, 2, T], MMDT)
            # V natural + 64 ones columns per head (denominator broadcast rows)
            v_sb = big.tile([128, 16, HPG, 128], MMDT)
            bq_sb = big.tile([128, 2], F32)
            bk_sb = big.tile([128, 2], F32)

            # DMA: ordered by first use. x split into 4 T-chunks so K proj
            # (and the first score tiles) start long before x fully lands.
            nc.sync.dma_start(out=wk_sb[:], in_=wk_ap)
            nc.sync.dma_start(out=wq_sb[:], in_=wq_ap)
            nc.sync.dma_start(out=bk_sb[:], in_=bk_ap)
            nc.sync.dma_start(out=bq_sb[:], in_=bq_ap)
            nc.sync.dma_start(out=xt_sb[:, :, 0:512], in_=xt_ap[:, :, 0:512])
            nc.sync.dma_start(out=wv_sb[:], in_=wv_ap)
            for tcn in range(1, 4):
                ts = slice(tcn * 512, (tcn + 1) * 512)
                nc.sync.dma_start(out=xt_sb[:, :, ts], in_=xt_ap[:, :, ts])
            nc.sync.dma_start(out=wo_sb[:], in_=wo_ap)
            nc.vector.memset(v_sb[:, :, :, HD:128], 1.0)

            def _proj_chunk(w_sb, b_sb, o_sb, jt, tcn):
                """QT/KT for feature tile jt, T-chunk tcn (512 cols)."""
                ts = slice(tcn * 512, (tcn + 1) * 512)
                p1 = psA.tile([128, 512], F32, tag="mm", name="p1")
                for cc in range(8):
                    nc.tensor.matmul(
                        p1[:],
                        w_sb[:, cc, jt * 128 : (jt + 1) * 128],
                        xt_sb[:, cc, ts],
                        start=(cc == 0),
                        stop=(cc == 7),
                    )
                nc.vector.tensor_scalar_add(
                    out=o_sb[:, jt, ts], in0=p1[:], scalar1=b_sb[:, jt : jt + 1]
                )

            def _k(jt, tcn):
                _proj_chunk(wk_sb, bk_sb, kt_sb, jt, tcn)

            def _q(jt, qb):
                _proj_chunk(wq_sb, bq_sb, qt_sb, jt, qb)

            def _v_chunk(tcn):
                for tg in (2 * tcn, 2 * tcn + 1):
                    pv = psA.tile([128, 1024], F32, tag="mm", name="pv")
                    for half in range(2):
                        tt = tg * 2 + half
                        for cc in range(8):
                            nc.tensor.matmul(
                                pv[:, half * 512 : half * 512 + JW],
                                xt_sb[:, cc, tt * 128 : (tt + 1) * 128],
                                wv_sb[:, cc, :],
                                start=(cc == 0),
                                stop=(cc == 7),
                            )
                    pv3 = pv[:].rearrange("p (half j) -> p half j", half=2)
                    nc.vector.tensor_copy(
                        out=v_sb[:, tg * 2 : tg * 2 + 2, :, 0:HD],
                        in_=pv3[:, :, 0:JW].rearrange(
                            "p half (h d) -> p half h d", h=HPG
                        ),
                    )

            es_tiles = {}

            def _scores(h, qb, kgs):
                """Score + exp tiles for head h, q-block qb, kg chunks kgs."""
                jt, pb = h // 2, 64 * (h % 2)
                qs = slice(qb * 512, (qb + 1) * 512)
                for kg in kgs:
                    st = psA.tile([128, 1024], F32, tag="mm", name="st")
                    for half in range(2):
                        kc = kg * 2 + half
                        nc.tensor.matmul(
                            st[:, half * 512 : (half + 1) * 512],
                            kt_sb[pb : pb + HD, jt, kc * 128 : (kc + 1) * 128],
                            qt_sb[pb : pb + HD, jt, qs],
                            start=True,
                            stop=True,
                        )
                    es = work.tile([128, 1024], MMDT, tag="es", name="es")
                    nc.scalar.activation(
                        out=es[:], in_=st[:], func=mybir.ActivationFunctionType.Exp
                    )
                    es_tiles[(h, qb, kg)] = es

            yext_tiles = {}

            def _pv(h, qb):
                yext = psY.tile([128, 512], F32, tag="yext", name="yext")
                for kg in range(8):
                    es = es_tiles.pop((h, qb, kg))
                    for half in range(2):
                        kc = kg * 2 + half
                        nc.tensor.matmul(
                            yext[:],
                            v_sb[:, kc, h, :],
                            es[:, half * 512 : (half + 1) * 512],
                            start=(kc == 0),
                            stop=(kc == 15),
                        )
                yext_tiles[(h, qb)] = yext

            def _norm(h, qb):
                jt, pb = h // 2, 64 * (h % 2)
                qs = slice(qb * 512, (qb + 1) * 512)
                yext = yext_tiles.pop((h, qb))
                r32 = nrm.tile([HD, 512], F32, tag="r32", name="r32")
                nc.vector.reciprocal(out=r32[:], in_=yext[HD:128, :])
                nc.vector.tensor_mul(
                    out=yt_sb[pb : pb + HD, jt, qs], in0=r32[:], in1=yext[0:HD, :]
                )

            def _outproj(tts):
                for tt in tts:
                    po = psA.tile([128, 1024], F32, tag="mm", name="po")
                    for mb in range(2):
                        for jt in range(2):
                            nc.tensor.matmul(
                                po[:, mb * 512 : (mb + 1) * 512],
                                yt_sb[:, jt, tt * 128 : (tt + 1) * 128],
                                wo_sb[:, jt, mb * 512 : (mb + 1) * 512],
                                start=(jt == 0),
                                stop=(jt == 1),
                            )
                    ob = outp.tile([128, 1024], MMDT, tag="ob", name="ob")
                    nc.vector.tensor_copy(out=ob[:], in_=po[:])
                    nc.sync.dma_start(out=out[tt * 128 : (tt + 1) * 128, :], in_=ob[:])

            # ---- lead-in: start the exp stream as early as possible ----
            _k(0, 0)
            _q(0, 0)
            _scores(0, 0, [0, 1])
            _k(0, 1)
            _scores(0, 0, [2, 3])
            _k(0, 2)
            _scores(0, 0, [4, 5])
            _k(0, 3)
            _scores(0, 0, [6, 7])
            _q(1, 0)
            _v_chunk(0)
            _v_chunk(1)
            _k(1, 0)
            _k(1, 1)
            _v_chunk(2)
            _v_chunk(3)

            # iteration i = (qb, h), qb-major; emission: S(i+1), PV(i), N(i),
            # fill(i). A resource consumed by S(j) must be emitted at
            # fill(j-2) or earlier since S(j) is emitted before fill(j-1).
            fillers = {
                0: [lambda: (_k(1, 2), _k(1, 3))],
                2: [lambda: _q(0, 1)],
                3: [lambda: _outproj([0, 1])],
                4: [lambda: (_q(1, 1), _outproj([2, 3]))],
                6: [lambda: _q(0, 2)],
                7: [lambda: _outproj([4, 5])],
                8: [lambda: (_q(1, 2), _outproj([6, 7]))],
                10: [lambda: _q(0, 3)],
                11: [lambda: _outproj([8, 9])],
                12: [lambda: (_q(1, 3), _outproj([10, 11]))],
                15: [lambda: _outproj([12, 13])],
            }
            iters = [(qb, h) for qb in range(4) for h in range(HPG)]
            for i in range(16):
                if i + 1 < 16:
                    qb1, h1 = iters[i + 1]
                    _scores(h1, qb1, range(8))
                qb0, h0 = iters[i]
                _pv(h0, qb0)
                _norm(h0, qb0)
                for f in fillers.get(i, []):
                    f()
            _outproj([14, 15])

            if debug:
                dq = nc.dram_tensor("dbg_q", [128, 2, T], MMDT, kind="ExternalOutput")
                dk = nc.dram_tensor("dbg_k", [128, 2, T], MMDT, kind="ExternalOutput")
                dy = nc.dram_tensor("dbg_y", [128, 2, T], MMDT, kind="ExternalOutput")
                dv = nc.dram_tensor(
                    "dbg_v", [128, 16, HPG, 128], MMDT, kind="ExternalOutput"
                )
                nc.sync.dma_start(out=dq[:, :, :], in_=qt_sb[:])
                nc.sync.dma_start(out=dk[:, :, :], in_=kt_sb[:])
                nc.sync.dma_start(out=dy[:, :, :], in_=yt_sb[:])
                nc.sync.dma_start(out=dv[:, :, :, :], in_=v_sb[:])

    nc.finalize()
    return nc


def _get_nc():
    global _CACHED_NC
    if _CACHED_NC is None:
        _CACHED_NC = _build()
    return _CACHED_NC


def make_in_maps(x, Wq, bq, Wk, bk, Wv, Wo):
    """Per-core input dicts (host-side sharding + layout + fp16 cast)."""
    xts = [
        np.ascontiguousarray(np.asarray(x[b], np.float32).T).astype(NPDT)
        for b in range(B)
    ]
    wq_f = np.asarray(Wq, np.float32) / 8.0
    wk_f = np.asarray(Wk, np.float32)
    wv_f = np.asarray(Wv, np.float32)
    wo_f = np.asarray(Wo, np.float32)
    bq_f = np.asarray(bq, np.float32) / 8.0
    bk_f = np.asarray(bk, np.float32)
    in_maps = []
    for c in range(NCORES):
        b, g = c // GROUPS, c % GROUPS
        js = slice(g * JW, (g + 1) * JW)
        in_maps.append(
            {
                "xt": xts[b],
                "wq": np.ascontiguousarray(wq_f[:, js]).astype(NPDT),
                "wk": np.ascontiguousarray(wk_f[:, js]).astype(NPDT),
                "wv": np.ascontiguousarray(wv_f[:, js]).astype(NPDT),
                "wo": np.ascontiguousarray(wo_f[js, :]).astype(NPDT),
                "bq": np.ascontiguousarray(bq_f[js]),
                "bk": np.ascontiguousarray(bk_f[js]),
            }
        )
    return in_maps


def combine(results, bias_row):
    """Sum per-core head-group partials and add the host-side bias row."""
    out = np.zeros((B, T, C), np.float32)
    for c in range(NCORES):
        out[c // GROUPS] += results[c]["out"].astype(np.float32)
    out += bias_row
    return out


def kernel(x, Wq, bq, Wk, bk, Wv, bv, Wo, bo):
    nc = _get_nc()
    in_maps = make_in_maps(x, Wq, bq, Wk, bk, Wv, Wo)
    res = run_bass_kernel_spmd(nc, in_maps, core_ids=list(range(NCORES)))
    bias_row = (
        np.asarray(bv, np.float32) @ np.asarray(Wo, np.float32)
        + np.asarray(bo, np.float32)
    ).astype(np.float32)
    return combine(res.results, bias_row)
